# revision 63
# baseline (speedup 1.0000x reference)
import sys

import numpy as np
import ml_dtypes

sys.path.insert(0, "/opt/trn_rl_repo")

import concourse.bass as bass  # noqa: E402
import concourse.bacc as bacc  # noqa: E402
import concourse.tile as tile  # noqa: E402
from concourse.tile import add_dep_helper  # noqa: E402
from concourse import mybir  # noqa: E402
from concourse.bass_utils import run_bass_kernel_spmd  # noqa: E402

# Problem dims (hardcoded per spec)
N, T, V, C_IN, C_OUT, K, KT = 256, 2048, 9, 16, 3, 5, 9
F_IN = V * C_IN    # 144
F_OUT = V * C_OUT  # 27
N_CORES = 8
NS = N // N_CORES  # 32 samples per core

WIN = 120          # conv output columns per window
NW = 18            # windows: 17*120 + 8 = 2048
WC = 128           # zT window length (WIN + 8 halo)
W17 = 16           # cols per sample in packed window 17 (12 data + 4 zero)
S36 = NS + 4       # giant col stride per t: 32 pose slots + 4 x2-group slots
GT = 1928          # giant t-span: t in [116, 2044)
GCOLS = S36 * GT   # 69408: t-major, no halo duplication
W0COLS = 5184      # w0x: window-0 (4608, sample-major) + window-17 (576)

F32 = mybir.dt.float32
F16 = mybir.dt.float16
F8 = mybir.dt.float8e3
E3 = ml_dtypes.float8_e3m4

_PROGRAM_CACHE = {}


def _build_program(warm=40, ztb=4, psg=3, psc=4, headq='sp', tailq=18, alt=False, late32=0, gate=0, xinb=NW-1, osbb=NW, shift=1, split_cf=1, fills=(20, 0, 18, 0, 10), c17_last=0):
    nc = bacc.Bacc()

    # t-major giant input: col = S36*(t-116) + slot; slots 0:32 pose
    # (partition = feat 0..127), slots 32:36 x2 groups (partition =
    # 16*(s%8) + feat-128). Covers windows 1..16 with zero halo overhead.
    giant = nc.declare_dram_parameter("giant", [128, GCOLS], F8, isOutput=False)
    # sample-major windows 0 and 17 (umambiguous early start / tail)
    w0x = nc.declare_dram_parameter("w0x", [128, W0COLS], F8, isOutput=False)
    # packed consts: f16 = weff1(0:27) | w2big(27:243) | ball(243:1323)
    cf16 = nc.declare_dram_parameter("cf16", [128, 1323], F16, isOutput=False)
    # single-row consts: beff(0:432) | btcn(432:435); broadcast on device
    craw = nc.declare_dram_parameter("craw", [1, 435], F16, isOutput=False)
    # raw dump: [window, time-in-window, 27*s + 3*w + o']; host unpacks
    out = nc.declare_dram_parameter("out", [NW, WIN, 27 * NS], F16, isOutput=True)

    with tile.TileContext(nc) as tc:
        with (
            tc.tile_pool(name="const", bufs=1) as cpool,
            tc.tile_pool(name="zt", bufs=ztb) as ztp,
            tc.tile_pool(name="osb", bufs=osbb) as osp,
            tc.tile_pool(name="psG", bufs=psg, space=bass.MemorySpace.PSUM) as psG,
            tc.tile_pool(name="psC", bufs=psc, space=bass.MemorySpace.PSUM) as psC,
            tc.tile_pool(name="psF", bufs=1, space=bass.MemorySpace.PSUM) as psF,
        ):
            cf16_sb = cpool.tile([128, 1323], F16, tag="cf16")
            cf32_sb = cpool.tile([128, 435], F16, tag="cf32")
            craw_sb = cpool.tile([1, 435], F16, tag="craw")
            ones_sb = cpool.tile([1, 128], F16, tag="ones")
            g_sb = cpool.tile([128, GCOLS], F8, tag="giant")
            w0x_sb = cpool.tile([128, W0COLS], F8, tag="w0x")
            weff1_sb = cf16_sb[:, 0:27]
            w2big_sb = cf16_sb[:, 27:243]
            ball_sb = cf16_sb[:, 243:1323]
            beff_sb = cf32_sb[:, 0:432]
            btcn_sb = cf32_sb[:, 432:435]

            # PE warmup: keep the tensor engine busy (and the p-state ramp
            # running) while the first input/const DMAs are in flight
            wmt = cpool.tile([128, 64], F16, tag="wmt")
            nc.vector.memset(wmt[:], 0.0)
            nc.vector.memset(ones_sb[:], 1.0)
            psw = psF.tile([128, 64], F32, tag="f", name="warm")
            for i in range(warm):
                nc.tensor.matmul(psw[0:64, 0:64], wmt[:], wmt[:],
                                 start=True, stop=True)

            def filler(n):
                # dependency-free matmuls that keep the PE p-state ramp
                # alive while the next real block's input DMA lands
                for i in range(n):
                    nc.tensor.matmul(psw[0:64, 0:64], wmt[:], wmt[:],
                                     start=True, stop=True)

            nc.scalar.dma_start(craw_sb[:], craw[:])
            if split_cf:
                # weff+w2big first (fast GCN start), ball after the inputs
                # the first windows need
                nc.sync.dma_start(cf16_sb[:, 0:256], cf16[:, 0:256])
            else:
                nc.sync.dma_start(cf16_sb[:], cf16[:])
            nc.sync.dma_start(w0x_sb[:, 4608:5184], w0x[:, 4608:5184])
            nc.sync.dma_start(w0x_sb[:, 0:2304], w0x[:, 0:2304])
            nc.sync.dma_start(w0x_sb[:, 2304:4608], w0x[:, 2304:4608])
            if split_cf == 1:
                nc.sync.dma_start(cf16_sb[:, 256:1323], cf16[:, 256:1323])
            xdmas = []
            gchunks = [0] + [4608 + 4320 * i for i in range(16)]
            for ci in range(len(gchunks) - 1):
                a, b = gchunks[ci], gchunks[ci + 1]
                xdmas.append(nc.sync.dma_start(g_sb[:, a:b], giant[:, a:b]))
                if split_cf == 2 and ci == 0:
                    nc.sync.dma_start(cf16_sb[:, 256:1323], cf16[:, 256:1323])

            # broadcast single-row consts (beff | btcn) to 128 partitions
            psB = psG.tile([128, 435], F32, tag="g", name="bcast")
            nc.tensor.matmul(psB[0:128, 0:435], ones_sb[:], craw_sb[:],
                             start=True, stop=True)
            nc.vector.tensor_scalar(out=cf32_sb[:], in0=psB[0:128, :],
                                    scalar1=1.0, scalar2=None,
                                    op0=mybir.AluOpType.mult)

            def gcn_lhsT(k, col, ncols):
                # lhsT [128, ncols] for slot `col` (sample s or x2 group 32+g)
                if k == 0:
                    if col < NS:
                        g, b = col // 8, col % 8
                        return w0x_sb[:, 1152 * g + 128 * b:1152 * g + 128 * b + 128]
                    g = col - NS
                    return w0x_sb[:, 1152 * g + 1024:1152 * g + 1152]
                if k == NW - 1:
                    if col < NS:
                        return w0x_sb[:, 4608 + 16 * col:4608 + 16 * col + 16]
                    return w0x_sb[:, 5120 + 16 * (col - NS):5120 + 16 * (col - NS) + 16]
                base = 4320 * (k - 1) + col
                return g_sb[:, base:base + S36 * (ncols - 1) + 1:S36]

            zts = [None] * NW

            def emit_gcn(k):
                last = k == NW - 1
                cw = W17 if last else WC   # t-cols in this window
                nt = 12 if last else 128   # valid zT rows (time positions)
                # window 17's zt must survive until its conv at the very end
                if last and c17_last:
                    zt = cpool.tile([128, 27 * NS], F16, tag="zt17")
                else:
                    zt = ztp.tile([128, 27 * NS], F16, tag="zt")
                zts[k] = zt
                if last:
                    nc.vector.memset(zt[:], 0.0)  # rows >= 12 stay 0 (t >= T)
                for h in range(2):  # 16 samples per psum bank
                    ps = psG.tile([128, 432], F32, tag="g")
                    for sl in range(16):
                        s = 16 * h + sl
                        nc.tensor.matmul(
                            ps[0:cw, 27 * sl:27 * sl + 27],
                            gcn_lhsT(k, s, cw), weff1_sb,
                            start=(sl == 0), stop=False,
                        )
                    for g2 in range(2):
                        g = 2 * h + g2
                        nc.tensor.matmul(
                            ps[0:cw, 216 * g2:216 * g2 + 216],
                            gcn_lhsT(k, NS + g, cw), w2big_sb,
                            start=False, stop=(g2 == 1),
                        )
                    nc.vector.tensor_tensor(
                        zt[0:nt, 432 * h:432 * h + 432], ps[0:nt, :],
                        beff_sb[0:nt, :], mybir.AluOpType.add,
                    )
                if k == 0:
                    nc.gpsimd.memset(zt[0:4, :], 0.0)   # z[t<0] = 0

            def emit_conv(k):
                last = k == NW - 1
                zt = zts[k]
                # conv: out[120k + i, (w, o')] via banded-Toeplitz stationary
                ot = osp.tile([128, 27 * NS], F16, tag="osb")
                for op_ in range(3):
                    pc = psC.tile([128, 9 * NS], F32, tag="c")
                    for o in range(3):
                        q = 3 * o + op_
                        nc.tensor.matmul(
                            pc[0:WIN, :],
                            ball_sb[:, WIN * q:WIN * q + WIN],
                            zt[:, o:27 * NS:3],
                            start=(o == 0), stop=(o == 2),
                        )
                    nc.scalar.activation(
                        ot[0:WIN, op_:27 * NS:3], pc[0:WIN, :],
                        mybir.ActivationFunctionType.Lrelu,
                        bias=btcn_sb[0:WIN, op_:op_ + 1], alpha=0.01,
                    )
                nr = 8 if last else WIN
                if not last and k >= NW - 1 - tailq:
                    oeng = (nc.sync if (NW - 1 - k) % 2 == (1 if alt else 0)
                            else nc.gpsimd) if alt else nc.sync
                else:
                    oeng = nc.gpsimd
                oi = oeng.dma_start(out[k, 0:nr], ot[0:nr, :])
                if gate and not last:
                    add_dep_helper(oi.ins, xdmas[gate].ins,
                                   reason="defer outs behind input stream")

            # software pipeline: emit conv one window behind GCN so the
            # conv never waits on the just-issued evacuations. `fills[i]`
            # PE filler matmuls after the i-th emission block bridge early
            # input-DMA latency without letting the PE ramp reset.
            step = 0

            def fill_after():
                nonlocal step
                if step < len(fills):
                    filler(fills[step])
                step += 1

            seq = [NW - 1] + list(range(NW - 1))
            for idx, k in enumerate(seq):
                emit_gcn(k)
                fill_after()
                if shift and idx >= shift:
                    j = seq[idx - shift]
                    if not (c17_last and j == NW - 1):
                        emit_conv(j)
                    fill_after()
                elif not shift:
                    emit_conv(k)
                    fill_after()
            if shift:
                for idx in range(NW - shift, NW):
                    emit_conv(seq[idx])
            if c17_last:
                # window 17's conv last: its 8-row output is the only DMA
                # whose pipeline latency is serially exposed at the end
                emit_conv(NW - 1)

    nc.finalize()
    return nc


def _host_consts(A, W_gcn, b_gcn, W_tcn, b_tcn):
    A = np.asarray(A, np.float32)
    W_gcn = np.asarray(W_gcn, np.float32)
    b_gcn = np.asarray(b_gcn, np.float32)
    W_tcn = np.asarray(W_tcn, np.float32)
    b_tcn = np.asarray(b_tcn, np.float32)

    # W_eff[(v,c),(w,o)] = sum_k W_gcn[k,o,c] A[k,v,w]
    W_eff = np.einsum("koc,kvw->vcwo", W_gcn, A).reshape(F_IN, F_OUT)
    b_eff = np.einsum("ko,kw->wo", b_gcn, A.sum(axis=1)).reshape(F_OUT)

    cf16 = np.zeros((128, 1323), np.float16)
    cf16[:, 0:27] = W_eff[:128]
    for sm in range(8):
        cf16[16 * sm:16 * sm + 16, 27 + 27 * sm:27 + 27 * sm + 27] = W_eff[128:144]
    # conv taps: out[t,(w,o')] = sum_tau sum_o Ctaps[tau][o,o'] z[t+tau,(w,o)]
    Ctaps = {tau: W_tcn[:, :, 4 - tau, 0].T for tau in range(-4, 5)}
    ii = np.arange(WIN)
    for o in range(3):
        for op_ in range(3):
            q = 3 * o + op_
            for d in range(-4, 5):
                cf16[ii + d + 4, 243 + WIN * q + ii] = Ctaps[d][o, op_]

    craw = np.zeros((1, 435), np.float16)
    craw[0, 0:432] = np.tile(b_eff, 16)
    craw[0, 432:435] = b_tcn
    return cf16, craw


def _host_windows(pose):
    # pose [N, T, 144] f32 -> per-core (w0x sample-major, giant t-major) fp8
    x8 = np.ascontiguousarray(pose.transpose(0, 2, 1)).astype(E3)  # [N, 144, T]
    Q = np.zeros((N, F_IN, 2056), E3)
    Q[:, :, 4:4 + T] = x8   # t index = col - 4; t in [-4, 2052)
    w0s, giants = [], []
    for c in range(N_CORES):
        S = Q[NS * c:NS * c + NS]            # [32, 144, 2056]
        w0 = np.zeros((128, W0COLS), E3)
        for g in range(4):
            blk = S[8 * g:8 * g + 8]         # [8, 144, 2056]
            w0[:, 1152 * g:1152 * g + 1024] = (
                blk[:, :128, 0:128].transpose(1, 0, 2).reshape(128, 1024))
            w0[:, 1152 * g + 1024:1152 * g + 1152] = (
                blk[:, 128:144, 0:128].reshape(128, 128))
        # window-17 region: t in [2036, 2052) -> Q cols [2040, 2056)
        w0[:, 4608:5120] = (
            S[:, :128, 2040:2056].transpose(1, 0, 2).reshape(128, 512))
        w17x2 = S[:, 128:144, 2040:2056].reshape(4, 8, 16, 16)
        w0[:, 5120:5184] = w17x2.transpose(1, 2, 0, 3).reshape(128, 64)
        view = S[:, :, 120:120 + GT]         # t in [116, 2044)
        G = np.zeros((128, GT, S36), E3)
        G[:, :, 0:NS] = view[:, :128, :].transpose(1, 2, 0)
        B2 = view[:, 128:144, :].reshape(4, 8, 16, GT)
        G[:, :, NS:S36] = B2.transpose(1, 2, 3, 0).reshape(128, GT, 4)
        w0s.append(w0)
        giants.append(G.reshape(128, GCOLS))
    return w0s, giants


def _run(inputs, **spmd_kwargs):
    pose = np.asarray(inputs["pose_feats"], np.float32)
    w0s, giants = _host_windows(pose)
    cf16, craw = _host_consts(
        inputs["A"], inputs["W_gcn"], inputs["b_gcn"], inputs["W_tcn"], inputs["b_tcn"]
    )

    if "prog" not in _PROGRAM_CACHE:
        _PROGRAM_CACHE["prog"] = _build_program()
    nc = _PROGRAM_CACHE["prog"]

    in_maps = []
    for i in range(N_CORES):
        in_maps.append({
            "w0x": w0s[i], "giant": giants[i], "cf16": cf16, "craw": craw,
        })
    res = run_bass_kernel_spmd(nc, in_maps, list(range(N_CORES)), **spmd_kwargs)
    outs = [res.results[i]["out"] for i in range(N_CORES)]
    full = np.stack(outs, axis=0)                 # [8, 18, 120, 864]
    full = full.reshape(N_CORES, NW, WIN, NS, F_OUT)
    full = full.transpose(0, 3, 1, 2, 4).reshape(N, NW * WIN, F_OUT)
    return full[:, :T].astype(np.float32), res


def kernel(**inputs) -> np.ndarray:
    out, _ = _run(inputs)
    return out

